# revision 1
# baseline (speedup 1.0000x reference)
"""Causal self-attention (GQA + RoPE + QK-norm) Trainium2 Bass kernel.

Sharding: 8 cores = 4 batches x 2 head-groups.  Core c -> batch c//2,
q heads (c%2)*8..+8, kv heads (c%2)*2..+2.  wproj is row-sharded, so each
core emits a partial (T, C) output; the host sums the two partials per batch.

Device-side layout strategy (per core):
  - x is fed pre-transposed (xT, [C, T]) and bf16-cast by the host.
  - QKV projections produce Q,K token-major ([tok, cols]); RoPE + rms-norm
    run token-major (free-axis per-head reductions), then 128x128 PE
    transposes produce qT/kT feature-major for the attention matmuls.
    V is produced token-major, which is exactly the p@v stationary layout.
  - scores are computed transposed (scoresT[tk, tq]) so that after exp the
    p tiles are already the moving operand for the p@v matmul; the softmax
    denominator comes from a ones-column matmul accumulated in PSUM.
  - exp has no max-subtraction: qk-norm bounds |s| <= sqrt(128) ~ 11.32.
  - output projection accumulates over the 8 local heads; partial written
    fp32 to DRAM.
"""

import numpy as np
import ml_dtypes
from contextlib import ExitStack

import concourse.bass as bass
import concourse.mybir as mybir
import concourse.tile as tile
from concourse import bacc
from concourse.bass_utils import run_bass_kernel_spmd
from concourse.masks import make_identity

BF16 = mybir.dt.bfloat16
F32 = mybir.dt.float32
F32R = mybir.dt.float32r
AF = mybir.ActivationFunctionType

B, T, C = 4, 2048, 2048
H, KV, D = 16, 4, 128
HG, KVG = H // 2, KV // 2          # per-core q heads (8), kv heads (2)
QC, KC = HG * D, KVG * D           # 1024, 256
P = 128
TOKCH = T // P                     # 16 token chunks
NREP = H // KV                     # 4
EPS = 1e-5
NEG = -1.0e5                       # additive causal mask (exp -> 0)


DEBUG_DUMP = False
PHASES = ("A", "B", "C")


def _build():
    nc = bacc.Bacc("TRN2", target_bir_lowering=False, debug=False, num_devices=8)
    xt = nc.dram_tensor("xt", [C, T], BF16, kind="ExternalInput")
    wq = nc.dram_tensor("wq", [C, QC], BF16, kind="ExternalInput")
    wkv = nc.dram_tensor("wkv", [C, 2 * KC], BF16, kind="ExternalInput")
    wp = nc.dram_tensor("wp", [QC, C], BF16, kind="ExternalInput")
    cosd = nc.dram_tensor("cosd", [T, D // 2], F32, kind="ExternalInput")
    sind = nc.dram_tensor("sind", [T, D // 2], F32, kind="ExternalInput")
    out = nc.dram_tensor("out", [T, C], F32, kind="ExternalOutput")
    if DEBUG_DUMP:
        d_qt = nc.dram_tensor("d_qt", [P, HG, T], F32, kind="ExternalOutput")
        d_kt = nc.dram_tensor("d_kt", [P, KVG, T], F32, kind="ExternalOutput")
        d_v = nc.dram_tensor("d_v", [P, TOKCH, KC], F32, kind="ExternalOutput")
        d_yt = nc.dram_tensor("d_yt", [P, HG, T], F32, kind="ExternalOutput")

    with tile.TileContext(nc) as tc, ExitStack() as ctx:
        singles = ctx.enter_context(tc.tile_pool(name="singles", bufs=1))

        # ---- resident tensors ----
        wq_sb = singles.tile([P, C // P, QC], BF16)
        wkv_sb = singles.tile([P, C // P, 2 * KC], BF16)
        wqr = wq.rearrange("(co p) q -> p co q", p=P)
        wkvr = wkv.rearrange("(co p) q -> p co q", p=P)
        for co in range(C // P):
            nc.sync.dma_start(wq_sb[:, co, :], wqr[:, co, :])
            nc.sync.dma_start(wkv_sb[:, co, :], wkvr[:, co, :])
        cos_sb = singles.tile([P, TOKCH, D // 2], F32)
        nc.sync.dma_start(cos_sb, cosd.rearrange("(tc p) d -> p tc d", p=P))
        sin_sb = singles.tile([P, TOKCH, D // 2], F32)
        nc.sync.dma_start(sin_sb, sind.rearrange("(tc p) d -> p tc d", p=P))

        ident = singles.tile([P, P], BF16)
        make_identity(nc, ident)
        ones_col = singles.tile([P, 1], BF16)
        nc.vector.memset(ones_col, 1.0)
        ones_row = singles.tile([1, P], F32)
        nc.vector.memset(ones_row, 1.0)
        zero_col = singles.tile([P, 1], F32)
        nc.vector.memset(zero_col, 0.0)
        eps_col = singles.tile([P, 1], F32)
        nc.vector.memset(eps_col, EPS)
        nc.const_aps.aps[(F32, 0.0)] = zero_col[:]
        nc.const_aps.aps[(F32, EPS)] = eps_col[:]

        # 4 diagonal-block masks: variant o (offset o*128): keep where
        # i >= j + o*128  (j = tk partition, i = tq free)
        mask_sb = singles.tile([P, 4, 512], F32)
        nc.vector.memset(mask_sb, 0.0)
        for o in range(4):
            nc.gpsimd.affine_select(
                out=mask_sb[:, o, :], in_=mask_sb[:, o, :],
                compare_op=mybir.AluOpType.is_ge, fill=NEG,
                base=-o * P, pattern=[[1, 512]], channel_multiplier=-1,
            )

        qT = singles.tile([P, HG, T], BF16)      # [d, h, tok]
        kT = singles.tile([P, KVG, T], BF16)
        v_sb = singles.tile([P, TOKCH, KC], BF16)  # [tok%128, chunk, vcol]
        yT = singles.tile([P, HG, T], BF16)

        # ================= phase A: QKV proj + RoPE + qk-norm =============
        if "A" not in PHASES:
            pass
        else:
         with tc.tile_pool(name="xa", bufs=3) as xpool, \
             tc.tile_pool(name="pa", bufs=2, space="PSUM") as pps, \
             tc.tile_pool(name="sa", bufs=3) as spool:
            for t in range(TOKCH):
                xtile = xpool.tile([P, C // P, P], BF16, tag="xt")
                nc.sync.dma_start(xtile, xt.rearrange("(co p) t -> p co t", p=P)[:, :, t * P:(t + 1) * P])
                ps_q0 = pps.tile([P, 512], F32, tag="q0")
                ps_q1 = pps.tile([P, 512], F32, tag="q1")
                ps_kv = pps.tile([P, 512], F32, tag="kv")
                ps_k = ps_kv[:, 0:KC]
                ps_v = ps_kv[:, KC:2 * KC]
                nco = C // P
                for co in range(nco):
                    lhsT = xtile[:, co, :]
                    st = dict(start=(co == 0), stop=(co == nco - 1))
                    nc.tensor.matmul(ps_q0, lhsT, wq_sb[:, co, 0:512], **st)
                    nc.tensor.matmul(ps_q1, lhsT, wq_sb[:, co, 512:1024], **st)
                    nc.tensor.matmul(ps_kv, lhsT, wkv_sb[:, co, :], **st)

                # V: cast straight to resident token-major buffer
                nc.vector.tensor_copy(v_sb[:, t, :], ps_v)

                # Q/K: fused multi-head rope + rms-norm + cast + transpose
                def rope_norm(ps, nh, dstT, h0, qscale):
                    h2 = D // 2
                    v4 = ps.rearrange("p (h a d) -> p h a d", h=nh, a=2)
                    q1, q2 = v4[:, :, 0, :], v4[:, :, 1, :]
                    r = spool.tile([P, nh, 2, h2], F32, tag=f"rope{nh}")
                    r1, r2 = r[:, :, 0, :], r[:, :, 1, :]
                    s2 = spool.tile([P, nh, h2], F32, tag=f"scr{nh}")
                    cs = cos_sb[:, t, None, :].to_broadcast([P, nh, h2])
                    sn = sin_sb[:, t, None, :].to_broadcast([P, nh, h2])
                    nc.vector.tensor_mul(r1, q1, cs)
                    nc.vector.tensor_mul(s2, q2, sn)
                    nc.vector.tensor_sub(r1, r1, s2)
                    nc.vector.tensor_mul(r2, q1, sn)
                    nc.vector.tensor_mul(s2, q2, cs)
                    nc.vector.tensor_add(r2, r2, s2)
                    rf = r.rearrange("p h a d -> p h (a d)")
                    sq = spool.tile([P, nh, D], F32, tag=f"sq{nh}")
                    nc.scalar.activation(sq, rf, AF.Square)
                    ss = spool.tile([P, nh], F32, tag=f"ss{nh}")
                    nc.vector.tensor_reduce(ss, sq, axis=mybir.AxisListType.X,
                                            op=mybir.AluOpType.add)
                    rt = spool.tile([P, nh], F32, tag=f"rt{nh}")
                    nc.scalar.activation(rt, ss, AF.Sqrt, scale=1.0 / D, bias=EPS)
                    rq = spool.tile([P, nh], F32, tag=f"rq{nh}")
                    nc.vector.reciprocal(rq, rt)
                    if qscale != 1.0:
                        nc.vector.tensor_scalar_mul(rq, rq, qscale)
                    qbf = spool.tile([P, nh, D], BF16, tag=f"qbf{nh}")
                    nc.vector.tensor_mul(qbf, rf, rq[:, :, None].to_broadcast([P, nh, D]))
                    pst = pps.tile([P, 4, P], BF16, tag="tr")
                    for i in range(nh):
                        nc.tensor.transpose(pst[:, i, :], qbf[:, i, :], ident)
                    # one strided copy: psum [128, nh*128] -> nh head slices of dstT
                    nc.vector.tensor_copy(
                        dstT[:, h0:h0 + nh, t * P:(t + 1) * P], pst[:, 0:nh, :])

                qsc = 1.0 / float(np.sqrt(D))
                rope_norm(ps_q0, 4, qT, 0, qsc)
                rope_norm(ps_q1, 4, qT, 4, qsc)
                rope_norm(ps_k, KVG, kT, 0, 1.0)

        # ================= phase B: attention ============================
        if "B" not in PHASES:
            pass
        else:
         with tc.tile_pool(name="psc", bufs=4, space="PSUM") as psc, \
             tc.tile_pool(name="psy", bufs=2, space="PSUM") as psy, \
             tc.tile_pool(name="pss", bufs=2, space="PSUM") as pss, \
             tc.tile_pool(name="pb", bufs=4) as ppool, \
             tc.tile_pool(name="sb", bufs=3) as bpool:
            NT = T // 512  # 4 tq tiles
            for t in range(NT):
                for h in range(HG):
                    g = h // NREP
                    nch = 4 * (t + 1)
                    ps_y = psy.tile([P, 512], F32, tag="y")
                    ps_sden = pss.tile([P, 512], F32, tag="sden")
                    ps_s = ps_sden[0:1, :]
                    for c in range(nch):
                        o = c * P - t * 512
                        col0 = max(o, 0)
                        ps_sc = psc.tile([P, 512], F32, tag="sc")
                        nc.tensor.matmul(
                            ps_sc[:, col0:512], kT[:, g, c * P:(c + 1) * P],
                            qT[:, h, t * 512 + col0:(t + 1) * 512],
                            start=True, stop=True)
                        if o >= 0:
                            # after the col0 shift the partial block is always
                            # the i' >= j triangle
                            nc.vector.tensor_add(ps_sc[:, col0:col0 + P],
                                                 ps_sc[:, col0:col0 + P],
                                                 mask_sb[:, 0, 0:P])
                        pt = ppool.tile([P, 512], BF16, tag="pt")
                        nc.scalar.activation(pt[:, col0:512], ps_sc[:, col0:512], AF.Exp)
                        st = dict(start=(c == 0), stop=(c == nch - 1))
                        nc.tensor.matmul(ps_y[:, col0:512],
                                         v_sb[:, c, g * P:(g + 1) * P],
                                         pt[:, col0:512], **st)
                        nc.tensor.matmul(ps_s[:, col0:512], ones_col,
                                         pt[:, col0:512], **st)
                    rc = bpool.tile([1, 512], F32, tag="rc")
                    nc.vector.reciprocal(rc, ps_s)
                    nc.tensor.matmul(ps_sden, ones_row, rc, start=True, stop=True)
                    rb = bpool.tile([P, 512], F32, tag="rb")
                    nc.vector.tensor_copy(rb, ps_sden)
                    nc.vector.tensor_mul(yT[:, h, t * 512:(t + 1) * 512], ps_y, rb)

        if DEBUG_DUMP:
            with tc.tile_pool(name="dbg", bufs=2) as dpool:
                for h in range(HG):
                    dt_ = dpool.tile([P, T], F32, tag="d")
                    nc.vector.tensor_copy(dt_, qT[:, h, :])
                    nc.sync.dma_start(d_qt[:, h, :], dt_)
                    dt_ = dpool.tile([P, T], F32, tag="d")
                    nc.vector.tensor_copy(dt_, yT[:, h, :])
                    nc.sync.dma_start(d_yt[:, h, :], dt_)
                for g in range(KVG):
                    dt_ = dpool.tile([P, T], F32, tag="d")
                    nc.vector.tensor_copy(dt_, kT[:, g, :])
                    nc.sync.dma_start(d_kt[:, g, :], dt_)
                dt_ = dpool.tile([P, TOKCH * KC], F32, tag="d")
                nc.vector.tensor_copy(dt_.rearrange("p (a b) -> p a b", a=TOKCH), v_sb[:, :, :])
                nc.sync.dma_start(d_v[:, :, :], dt_.rearrange("p (a b) -> p a b", a=TOKCH))

        # ================= phase C: output projection =====================
        if "C" not in PHASES:
            pass
        else:
         with tc.tile_pool(name="wp", bufs=1) as wpool, \
             tc.tile_pool(name="po", bufs=2, space="PSUM") as pso, \
             tc.tile_pool(name="so", bufs=3) as opool:
            wpr = wp.rearrange("(hc p) c -> p hc c", p=P)
            wp_ts = []
            for ct in range(C // 512):
                wp_t = wpool.tile([P, HG, 512], BF16, tag=f"wpt{ct}")
                nc.sync.dma_start(wp_t, wpr[:, :, ct * 512:(ct + 1) * 512])
                wp_ts.append(wp_t)
            for t in range(TOKCH):
                for ct in range(C // 512):
                    ps_o = pso.tile([P, 512], F32, tag="o")
                    for hc in range(HG):
                        nc.tensor.matmul(
                            ps_o, yT[:, hc, t * P:(t + 1) * P], wp_ts[ct][:, hc, :],
                            start=(hc == 0), stop=(hc == HG - 1))
                    ob = opool.tile([P, 512], F32, tag="ob")
                    nc.vector.tensor_copy(ob, ps_o)
                    nc.sync.dma_start(out[t * P:(t + 1) * P, ct * 512:(ct + 1) * 512], ob)
    nc.compile()
    return nc


_NC_CACHE = []


def _get_prog():
    if not _NC_CACHE:
        _NC_CACHE.append(_build())
    return _NC_CACHE[0]


def _make_in_maps(inputs):
    x, cos, sin = inputs["x"], inputs["cos"], inputs["sin"]
    wq, wk, wv, wproj = inputs["wq"], inputs["wk"], inputs["wv"], inputs["wproj"]
    bf = ml_dtypes.bfloat16
    cos2 = np.ascontiguousarray(cos.reshape(T, D // 2), dtype=np.float32)
    sin2 = np.ascontiguousarray(sin.reshape(T, D // 2), dtype=np.float32)
    in_maps = []
    for core in range(8):
        b, g = core // 2, core % 2
        qs = slice(g * QC, (g + 1) * QC)
        ks = slice(g * KC, (g + 1) * KC)
        in_maps.append({
            "xt": np.ascontiguousarray(x[b].T).astype(bf),
            "wq": np.ascontiguousarray(wq[:, qs]).astype(bf),
            "wkv": np.ascontiguousarray(np.hstack([wk[:, ks], wv[:, ks]])).astype(bf),
            "wp": np.ascontiguousarray(wproj[qs, :]).astype(bf),
            "cosd": cos2,
            "sind": sin2,
        })
    return in_maps


def kernel(x, cos, sin, wq, wk, wv, wproj):
    nc = _get_prog()
    in_maps = _make_in_maps(dict(x=x, cos=cos, sin=sin, wq=wq, wk=wk, wv=wv, wproj=wproj))
    res = run_bass_kernel_spmd(nc, in_maps, core_ids=list(range(8))).results
    outp = np.empty((B, T, C), np.float32)
    for b in range(B):
        outp[b] = res[2 * b]["out"] + res[2 * b + 1]["out"]
    return outp



# revision 9
# speedup vs baseline: 1.2593x; 1.2593x over previous
"""Causal self-attention (GQA + RoPE + QK-norm) Trainium2 Bass kernel.

Sharding: 8 cores = 4 batches x 2 head-groups.  Core c -> batch c//2,
q heads (c%2)*8..+8, kv heads (c%2)*2..+2.  wproj is row-sharded, so each
core emits a partial (T, C) output; the host sums the two partials per batch.

Device-side layout (per core):
  - x fed pre-transposed (xT, [C, T]) bf16; cos|sin fed as one [T, 128] f32.
  - Phase A: QKV projection accumulates into one 3-bank PSUM tile per token
    chunk; RoPE + qk-norm run token-major on DVE (norm factors exact: RoPE
    preserves per-head L2 norm, and 1/sqrt(D) is folded into the q rsqrt via
    Rsqrt(ss + D*eps)); 128x128 PE transposes produce qT/kT feature-major.
  - Phase B: scores computed transposed (scoresT[tk, tq]); causal mask is a
    triangular NEG matmul accumulated into the same PSUM bank before the
    score matmul; exp (bias -4 so fp16 accumulators can't overflow) writes
    fp16 p tiles that feed the p@v matmul directly.  Softmax denominators
    accumulate on DVE (pacc) with a single ones-column matmul per head;
    1/denom is broadcast across partitions with an SBUF->SBUF DMA and applied
    with one DVE divide.  The per-head epilogue is software-pipelined one
    head behind to keep PE dense.
  - Phase C (output projection) is interleaved into phase B: the 512-token
    tile t's projection matmuls are emitted while tile t+1's attention runs,
    filling PE gaps left by the exp critical path.
"""

import numpy as np
import ml_dtypes
from contextlib import ExitStack

import concourse.bass as bass
import concourse.bass_isa as bass_isa
import concourse.mybir as mybir
import concourse.tile as tile
from concourse import bacc
from concourse.bass_utils import run_bass_kernel_spmd
from concourse.masks import make_identity

BF16 = mybir.dt.bfloat16
F16 = mybir.dt.float16
F32 = mybir.dt.float32
AF = mybir.ActivationFunctionType
ALU = mybir.AluOpType

B, T, C = 4, 2048, 2048
H, KV, D = 16, 4, 128
HG, KVG = H // 2, KV // 2          # per-core q heads (8), kv heads (2)
QC, KC = HG * D, KVG * D           # 1024, 256
P = 128
TOKCH = T // P                     # 16 token chunks
NREP = H // KV                     # 4
EPS = 1e-5
NEG = -1.0e5                       # additive causal mask (exp -> 0)
EXPB = -4.0                        # exp bias: pt <= e^{11.32-4} ~ 1.5e3; a
                                   # 16-chunk fp16 pacc sum stays < 2.5e4

PHASES = ("A", "B")


def _build():
    nc = bacc.Bacc("TRN2", target_bir_lowering=False, debug=False, num_devices=8)
    xt = nc.dram_tensor("xt", [C, T], BF16, kind="ExternalInput")
    wq = nc.dram_tensor("wq", [C, QC], BF16, kind="ExternalInput")
    wkv = nc.dram_tensor("wkv", [C, 2 * KC], BF16, kind="ExternalInput")
    wp = nc.dram_tensor("wp", [QC, C], BF16, kind="ExternalInput")
    cs = nc.dram_tensor("cs", [T, P], F32, kind="ExternalInput")
    out = nc.dram_tensor("out", [T, C], F32, kind="ExternalOutput")

    with tile.TileContext(nc) as tc, ExitStack() as ctx:
        singles = ctx.enter_context(tc.tile_pool(name="singles", bufs=1))
        xpool = ctx.enter_context(tc.tile_pool(name="xp", bufs=2))

        # ---- x prefetch first so PE can start ~immediately ----
        xr = xt.rearrange("(co p) t -> p co t", p=P)

        def load_x(t2):
            xtile = xpool.tile([P, C // P, 2 * P], BF16, tag="xt")
            nc.sync.dma_start(xtile, xr[:, :, t2 * 2 * P:(t2 + 1) * 2 * P])
            return xtile

        x_cur = load_x(0)

        # ---- weights (A-scoped pool so phase C's wp reuses the space) ----
        wqkvp = ctx.enter_context(tc.tile_pool(name="wqkv", bufs=1))
        wq_sb = wqkvp.tile([P, C // P, QC], BF16)
        wkv_sb = wqkvp.tile([P, C // P, 2 * KC], BF16)
        cs_sb = singles.tile([P, TOKCH, P], F32)
        wqr = wq.rearrange("(co p) q -> p co q", p=P)
        wkvr = wkv.rearrange("(co p) q -> p co q", p=P)
        for cb in range(4):
            s = slice(4 * cb, 4 * cb + 4)
            nc.sync.dma_start(wq_sb[:, s, :], wqr[:, s, :])
            nc.sync.dma_start(wkv_sb[:, s, :], wkvr[:, s, :])
            if cb == 0:
                nc.sync.dma_start(cs_sb, cs.rearrange("(tc p) d -> p tc d", p=P))

        ident = singles.tile([P, P], BF16)
        make_identity(nc, ident)
        for cval in (0.0, EPS, float(D) * EPS, EXPB):
            ccol = singles.tile([P, 1], F32, tag=f"c{cval}")
            nc.vector.memset(ccol, cval)
            nc.const_aps.aps[(F32, cval)] = ccol[:]

        # strict-upper triangular NEG (mask matmul stationary):
        # utri[p, m] = NEG if p < m else 0
        utri = singles.tile([P, P], BF16)
        nc.vector.memset(utri, 0.0)
        nc.gpsimd.affine_select(
            out=utri, in_=utri,
            compare_op=ALU.is_ge, fill=NEG,
            base=0, pattern=[[-1, P]], channel_multiplier=1,
        )

        qT = singles.tile([P, HG, T], BF16)      # [d, h, tok]
        kT = singles.tile([P, KVG, T], BF16)
        v_sb = singles.tile([P, TOKCH, KC], F16)  # [tok%128, chunk, vcol]
        yT = singles.tile([P, HG, T], BF16)

        # ================= phase A: QKV proj + RoPE + qk-norm =============
        if "A" in PHASES:
         with tc.tile_pool(name="pa", bufs=2, space="PSUM") as pps, \
             tc.tile_pool(name="pt2", bufs=2, space="PSUM") as ptp, \
             tc.tile_pool(name="sa", bufs=2) as spool, \
             tc.tile_pool(name="sb2", bufs=2) as qpool:
            NH = HG + KVG  # 10 rope heads
            h2 = D // 2
            for t in range(TOKCH):
                if t % 2 == 0 and t > 0:
                    x_cur = load_x(t // 2)
                xtile = x_cur[:, :, (t % 2) * P:(t % 2 + 1) * P]
                ps = pps.tile([P, QC + 2 * KC], F32, tag="qkv")  # 3 banks
                nco = C // P
                for co in range(nco):
                    lhsT = xtile[:, co, :]
                    st = dict(start=(co == 0), stop=(co == nco - 1))
                    nc.tensor.matmul(ps[:, 0:512], lhsT, wq_sb[:, co, 0:512], **st)
                    nc.tensor.matmul(ps[:, 512:1024], lhsT, wq_sb[:, co, 512:1024], **st)
                    nc.tensor.matmul(ps[:, 1024:1536], lhsT, wkv_sb[:, co, :], **st)

                # V: cast straight to resident token-major buffer (ACT)
                nc.scalar.copy(v_sb[:, t, :], ps[:, QC + KC:QC + 2 * KC])

                # RoPE (token-major, all 10 heads at once).
                # psum view: [P, NH, 2, h2] over q0..q7,k0,k1
                pv = ps[:, 0:QC + KC].rearrange("p (h a d) -> p h a d", h=NH, a=2)
                p1, p2 = pv[:, :, 0, :], pv[:, :, 1, :]
                r = spool.tile([P, NH, 2, h2], BF16, tag="r")
                r1, r2 = r[:, :, 0, :], r[:, :, 1, :]
                s2 = spool.tile([P, NH, h2], F32, tag="s2")
                csx = cs_sb[:, t, None, 0:h2].to_broadcast([P, NH, h2])
                snx = cs_sb[:, t, None, h2:P].to_broadcast([P, NH, h2])
                nc.vector.tensor_mul(r1, p1, csx)
                nc.vector.tensor_mul(s2, p2, snx)
                nc.vector.tensor_sub(r1, r1, s2)
                nc.vector.tensor_mul(r2, p1, snx)
                nc.vector.tensor_mul(s2, p2, csx)
                nc.vector.tensor_add(r2, r2, s2)

                # qk-norm factors (RoPE preserves per-head L2 norms, and it
                # is linear, so compute ss from r and scale r afterwards).
                rf = r.rearrange("p h a d -> p h (a d)")
                sq = qpool.tile([P, NH, D], BF16, tag="w")
                nc.vector.tensor_mul(sq, rf, rf)
                ss = spool.tile([P, NH], F32, tag="ss")
                nc.vector.tensor_reduce(ss, sq, axis=mybir.AxisListType.X,
                                        op=ALU.add)
                rt = spool.tile([P, NH], F32, tag="rt")
                # q heads: qsc/sqrt(ss/D+eps) == 1/sqrt(ss + D*eps)
                # (qsc = 1/sqrt(D) folded into the sqrt argument scale)
                nc.scalar.activation(rt[:, 0:HG], ss[:, 0:HG], AF.Sqrt,
                                     scale=1.0, bias=float(D) * EPS)
                nc.scalar.activation(rt[:, HG:NH], ss[:, HG:NH], AF.Sqrt,
                                     scale=1.0 / D, bias=EPS)
                rq = spool.tile([P, NH], F32, tag="rq")
                nc.vector.reciprocal(rq, rt)
                qk = qpool.tile([P, NH, D], BF16, tag="w")
                nc.vector.tensor_mul(
                    qk, rf, rq[:, :, None].to_broadcast([P, NH, D]))

                # transpose waves (5 heads each) -> qT/kT
                pst = ptp.tile([P, 5, P], BF16, tag="tr")
                for i in range(5):
                    nc.tensor.transpose(pst[:, i, :], qk[:, i, :], ident)
                nc.scalar.copy(qT[:, 0:5, t * P:(t + 1) * P], pst)
                pst = ptp.tile([P, 5, P], BF16, tag="tr")
                for i in range(5):
                    nc.tensor.transpose(pst[:, i, :], qk[:, 5 + i, :], ident)
                nc.scalar.copy(qT[:, 5:HG, t * P:(t + 1) * P], pst[:, 0:3, :])
                nc.scalar.copy(kT[:, :, t * P:(t + 1) * P], pst[:, 3:5, :])

        # ================= phase B+C: attention + interleaved out-proj ====
        if "B" in PHASES:
         with tc.tile_pool(name="wpp", bufs=1) as wpool, \
             tc.tile_pool(name="psc", bufs=4, space="PSUM") as psc, \
             tc.tile_pool(name="psy", bufs=2, space="PSUM") as psy, \
             tc.tile_pool(name="pso", bufs=2, space="PSUM") as pso, \
             tc.tile_pool(name="pb", bufs=3) as ppool, \
             tc.tile_pool(name="sb", bufs=2) as bpool:
            # wproj tiles: loaded at start of B into SBUF freed by wq/wkv
            wpr = wp.rearrange("(hc p) c -> p hc c", p=P)
            wp_ts = []
            for ct in range(C // 512):
                wp_t = wpool.tile([P, HG, 512], BF16, tag=f"wpt{ct}")
                nc.sync.dma_start(wp_t, wpr[:, :, ct * 512:(ct + 1) * 512])
                wp_ts.append(wp_t)

            NT = T // 512  # 4 tq tiles

            def proj_unit(t, u):
                """One phase-C psum tile: out[tok chunk, 512 c-cols]."""
                tc_, ct = t * 4 + u // 4, u % 4
                ps_o = pso.tile([P, 512], F32, tag="o")
                for hc in range(HG):
                    nc.tensor.matmul(
                        ps_o, yT[:, hc, tc_ * P:(tc_ + 1) * P],
                        wp_ts[ct][:, hc, :],
                        start=(hc == 0), stop=(hc == HG - 1))
                ob = bpool.tile([P, 512], F32, tag="ob")
                nc.vector.tensor_copy(ob, ps_o)
                nc.sync.dma_start(
                    out[tc_ * P:(tc_ + 1) * P, ct * 512:(ct + 1) * 512], ob)

            pend = None      # deferred per-head epilogue
            cqueue = []      # deferred phase-C units

            def emit_epilogue(e):
                ps_y, pacc, t, h = e
                # softmax denominator: cross-partition sum of pacc, result
                # replicated to all partitions (GpSimd is otherwise idle)
                rbb = bpool.tile([P, 512], F32, tag="rbb")
                nc.gpsimd.partition_all_reduce(
                    rbb, pacc, channels=P, reduce_op=bass_isa.ReduceOp.add)
                nc.vector.reciprocal(rbb, rbb)
                nc.vector.tensor_mul(
                    yT[:, h, t * 512:(t + 1) * 512], ps_y, rbb)

            for t in range(NT):
                for h in range(HG):
                    g = h // NREP
                    nch = 4 * (t + 1)
                    ps_y = psy.tile([P, 512], F32, tag="y")
                    pacc = ppool.tile([P, 512], F16, tag="pacc")
                    pts = {}

                    def score(c):
                        o = c * P - t * 512
                        col0 = max(o, 0)
                        ps_sc = psc.tile([P, 512], F32, tag="sc")
                        if o >= 0:
                            # causal mask: NEG upper triangle accumulated
                            # into the bank before the score matmul
                            nc.tensor.matmul(
                                ps_sc[:, col0:col0 + P], utri, ident,
                                start=True, stop=False)
                        nc.tensor.matmul(
                            ps_sc[:, col0:512], kT[:, g, c * P:(c + 1) * P],
                            qT[:, h, t * 512 + col0:(t + 1) * 512],
                            start=(o < 0), stop=True)
                        pt = ppool.tile([P, 512], F16, tag="pt")
                        nc.scalar.activation(pt[:, col0:512], ps_sc[:, col0:512],
                                             AF.Exp, bias=EXPB)
                        pts[c] = (pt, col0)

                    def ymm(c):
                        pt, col0 = pts.pop(c)
                        st = dict(start=(c == 0), stop=(c == nch - 1))
                        nc.tensor.matmul(ps_y[:, col0:512],
                                         v_sb[:, c, g * P:(g + 1) * P],
                                         pt[:, col0:512], **st)
                        if c == 0:
                            nc.vector.tensor_copy(pacc, pt)
                        else:
                            nc.vector.tensor_add(pacc[:, col0:512],
                                                 pacc[:, col0:512],
                                                 pt[:, col0:512])

                    # software-pipelined: y lags scores by 3
                    LAG = 3
                    for c in range(min(LAG, nch)):
                        score(c)
                    if pend is not None:
                        emit_epilogue(pend)
                        pend = None
                    for c in range(LAG, nch):
                        score(c)
                        ymm(c - LAG)
                    for c in range(max(nch - LAG, 0), nch):
                        ymm(c)
                    pend = (ps_y, pacc, t, h)

                    # phase-C filler for the previous tq tile
                    for _ in range(2):
                        if cqueue:
                            proj_unit(*cqueue.pop(0))
                cqueue.extend((t, u) for u in range(16))

            emit_epilogue(pend)
            for tu in cqueue:
                proj_unit(*tu)

    nc.compile()
    return nc


_NC_CACHE = []


def _get_prog():
    if not _NC_CACHE:
        _NC_CACHE.append(_build())
    return _NC_CACHE[0]


def _make_in_maps(inputs):
    x, cos, sin = inputs["x"], inputs["cos"], inputs["sin"]
    wq, wk, wv, wproj = inputs["wq"], inputs["wk"], inputs["wv"], inputs["wproj"]
    bf = ml_dtypes.bfloat16
    cos2 = np.asarray(cos.reshape(T, D // 2), dtype=np.float32)
    sin2 = np.asarray(sin.reshape(T, D // 2), dtype=np.float32)
    cs2 = np.ascontiguousarray(np.hstack([cos2, sin2]))
    in_maps = []
    for core in range(8):
        b, g = core // 2, core % 2
        qs = slice(g * QC, (g + 1) * QC)
        ks = slice(g * KC, (g + 1) * KC)
        in_maps.append({
            "xt": np.ascontiguousarray(x[b].T).astype(bf),
            "wq": np.ascontiguousarray(wq[:, qs]).astype(bf),
            "wkv": np.ascontiguousarray(np.hstack([wk[:, ks], wv[:, ks]])).astype(bf),
            "wp": np.ascontiguousarray(wproj[qs, :]).astype(bf),
            "cs": cs2,
        })
    return in_maps


def kernel(x, cos, sin, wq, wk, wv, wproj):
    nc = _get_prog()
    in_maps = _make_in_maps(dict(x=x, cos=cos, sin=sin, wq=wq, wk=wk, wv=wv, wproj=wproj))
    res = run_bass_kernel_spmd(nc, in_maps, core_ids=list(range(8))).results
    outp = np.empty((B, T, C), np.float32)
    for b in range(B):
        outp[b] = res[2 * b]["out"] + res[2 * b + 1]["out"]
    return outp
